# revision 79
# baseline (speedup 1.0000x reference)
"""Trainium2 Bass kernel for a talking-heads MHSA block.

Reference computation (B=4, P=2048, D=512, H=8, DF=64, fp32):
    q = (x @ Wq) / sqrt(DF);  k = x @ Wk;  v = x @ Wv      (per-head reshape)
    attn[b,h]   = q_h k_h^T
    attn2[b,g]  = sum_h Wtalk[g,h] attn[b,h]               (talking heads)
    P           = softmax(attn2 + bias, axis=-1)
    out         = concat_g(P_g v_g) @ Wo

Sharding: 8 cores, data-parallel: core c -> batch b=c//2, query-half s=c%2
(1024 query rows, all heads, full 2048 keys). No collectives.

Per-core algorithm (all-bf16 matmuls, fp32 accumulation, no transposes):
  - host pre-casts inputs to bf16, pre-transposes x -> x^T, and precomputes
    expb = exp(bias^T) in bf16
  - talking-heads mix folded into Q: S_mixed[g] = (Wtalk[g,h]/8 * Q) . K
    over all 512 features -> dense 512-deep bf16 matmul. (fp8 DoubleRow was
    measured 2x faster on the PE but fails the 2e-2 gate: e4m3 operand
    rounding gives ~4% logit noise -> rel err 4.7e-2; fp8 probs/V in AV
    give 3.2e-2. The talking-heads 8x redundancy is PE-conserved: per-head
    QK + on-chip mix needs a partition interleave no engine does cheaply.)
  - softmax via exp(S+b) = exp(S-2) * exp(b): ACT computes E=exp(S-2)
    straight from PSUM, DVE multiplies by expb (all-SBUF op, 2x DVE mode);
    the -2 shift cancels in normalization
  - AV bf16 with a ones-column in V' producing denominators for free; AV
    emission skewed 2 key-chunks behind the S-matmuls so the PE never waits
    on exp; qg(g+1) computed mid-g on DVE
  - normalization after AV; out^T blocks (split per g-pair so phase D deps
    are per-block) feed the output projection as lhsT directly.
"""
import sys
from contextlib import ExitStack

import numpy as np

if "/opt/trn_rl_repo" not in sys.path:
    sys.path.insert(0, "/opt/trn_rl_repo")

B, P, D = 4, 2048, 512
H, DF = 8, 64
G = H                 # output head groups
PH = P // 2           # query rows per core
DC = D // 128         # 4 contraction chunks for d
EC = (H * DF) // 128  # 4 chunks for e = (h, df)
QC = P // 128         # 16 key chunks
VW = DF + 4           # V' width per group: 64 cols of V + ones column + pad
                      # (dual-fp8 LoadWeights needs 4-byte-aligned tiles)
N_CORES = 8

_CACHE = {}
LAST_RESULTS = None


def _build_program():
    import concourse.mybir as mybir
    import concourse.tile as tile
    from concourse import bacc

    f32 = mybir.dt.float32
    bf16 = mybir.dt.bfloat16
    fp8 = mybir.dt.float8e4
    ACT = mybir.ActivationFunctionType
    DR = mybir.MatmulPerfMode.DoubleRow

    nc = bacc.Bacc("TRN2", target_bir_lowering=False, debug=False)
    xt = nc.dram_tensor("xt", [D, P], bf16, kind="ExternalInput").ap()
    xqt = nc.dram_tensor("xqt", [D, PH], bf16, kind="ExternalInput").ap()
    biast = nc.dram_tensor("biast", [G, P, PH], bf16, kind="ExternalInput").ap()
    wq = nc.dram_tensor("wq", [D, H * DF], bf16, kind="ExternalInput").ap()
    wk = nc.dram_tensor("wk", [D, H * DF], bf16, kind="ExternalInput").ap()
    wv = nc.dram_tensor("wv", [D, H * DF], bf16, kind="ExternalInput").ap()
    wo = nc.dram_tensor("wo", [H * DF, D], bf16, kind="ExternalInput").ap()
    wt = nc.dram_tensor("wt", [H * DF, G], f32, kind="ExternalInput").ap()
    y = nc.dram_tensor("y", [PH, D], bf16, kind="ExternalOutput").ap()

    with tile.TileContext(nc) as tc, ExitStack() as ctx:
        persist = ctx.enter_context(tc.tile_pool(name="persist", bufs=1))
        qt_sb = persist.tile([128, EC * PH], bf16, tag="qt")      # Q^T [e, p]
        kt_sb = persist.tile([128, EC * P], bf16, tag="kt")       # K^T [e, q]
        v_sb = persist.tile([128, QC * G * VW], bf16, tag="v")    # V' [q, g*VW+c]
        wo_sb = persist.tile([128, EC * D], bf16, tag="wo")
        wt_sb = persist.tile([128, EC * G], f32, tag="wt")
        # out^T [e, p] split per g-pair so phase D deps are per-block
        ocat_sbs = [persist.tile([128, PH], bf16, name=f"ocat{ec}", tag=f"ocat{ec}")
                    for ec in range(EC)]
        neg2_sb = persist.tile([128, 1], f32, tag="neg2")
        nc.gpsimd.memset(neg2_sb[:], -2.0)

        def bf_load(dst_tile, dram_ap, n, m):
            # bf16 DRAM -> SBUF, one descriptor per 128-partition chunk
            nc.gpsimd.dma_start(
                dst_tile[:].rearrange("p (c m) -> p c m", c=n),
                dram_ap.rearrange("(c p) m -> p c m", p=128))

        # ---------- phase B: staging + QKV projections ----------
        with ExitStack() as pb:
            stage = pb.enter_context(tc.tile_pool(name="stage", bufs=1))
            xt_sb = stage.tile([128, DC * P], bf16, tag="xt")
            xqt_sb = stage.tile([128, DC * PH], bf16, tag="xqt")
            wq_sb = stage.tile([128, DC * D], bf16, tag="wq")
            wk_sb = stage.tile([128, DC * D], bf16, tag="wk")
            wv_sb = stage.tile([128, DC * D], bf16, tag="wv")
            # Q-proj inputs split across BOTH queues in parallel (wq on
            # gpsimd, xqt on sync) so the first matmul starts earliest
            for dc in range(DC):
                nc.gpsimd.dma_start(wq_sb[:, dc * D:(dc + 1) * D],
                                    wq[dc * 128:(dc + 1) * 128, :])
            for dc in range(DC):
                nc.sync.dma_start(xqt_sb[:, dc * PH:(dc + 1) * PH],
                                  xqt[dc * 128:(dc + 1) * 128, :])
            for dc in range(DC):
                nc.sync.dma_start(wk_sb[:, dc * D:(dc + 1) * D],
                                  wk[dc * 128:(dc + 1) * 128, :])
            for dc in range(DC):
                nc.gpsimd.dma_start(xt_sb[:, dc * P:(dc + 1) * P],
                                    xt[dc * 128:(dc + 1) * 128, :])
            bf_load(wv_sb, wv, DC, D)
            bf_load(wo_sb, wo, EC, D)
            nc.sync.dma_start(
                wt_sb[:].rearrange("p (c m) -> p c m", c=EC),
                wt.rearrange("(c p) m -> p c m", p=128))

            nc.gpsimd.memset(v_sb[:], 1.0)  # ones columns of V'

            psA = pb.enter_context(tc.tile_pool(name="psA", bufs=2, space="PSUM"))
            psB = pb.enter_context(tc.tile_pool(name="psB", bufs=4, space="PSUM"))

            # Q^T[e, p] = Wq^T x^T (query half only)
            for ec in range(EC):
                q_ps = psA.tile([128, PH], f32, tag="qps")
                for pc in range(PH // 512):
                    for dc in range(DC):
                        nc.tensor.matmul(
                            q_ps[:, pc * 512:(pc + 1) * 512],
                            lhsT=wq_sb[:, dc * D + ec * 128: dc * D + (ec + 1) * 128],
                            rhs=xqt_sb[:, dc * PH + pc * 512: dc * PH + (pc + 1) * 512],
                            start=(dc == 0), stop=(dc == DC - 1))
                nc.scalar.activation(qt_sb[:, ec * PH:(ec + 1) * PH], q_ps[:], ACT.Copy)
            # K^T[e, q] over all keys (qn outer: early key-chunks complete
            # first so phase C's S-matmuls can begin before V-proj drains)
            for qn in range(P // 512):
                for ec in range(EC):
                    k_ps = psB.tile([128, 512], f32, tag="kvps")
                    for dc in range(DC):
                        nc.tensor.matmul(
                            k_ps[:],
                            lhsT=wk_sb[:, dc * D + ec * 128: dc * D + (ec + 1) * 128],
                            rhs=xt_sb[:, dc * P + qn * 512: dc * P + (qn + 1) * 512],
                            start=(dc == 0), stop=(dc == DC - 1))
                    nc.scalar.activation(
                        kt_sb[:, ec * P + qn * 512: ec * P + (qn + 1) * 512],
                        k_ps[:], ACT.Copy)
            # V[q, e] natural layout, scattered into V' with ones columns
            for qc in range(QC):
                v_ps = psB.tile([128, 512], f32, tag="kvps")
                for dc in range(DC):
                    nc.tensor.matmul(
                        v_ps[:],
                        lhsT=xt_sb[:, dc * P + qc * 128: dc * P + (qc + 1) * 128],
                        rhs=wv_sb[:, dc * D:(dc + 1) * D],
                        start=(dc == 0), stop=(dc == DC - 1))
                dst = v_sb[:, qc * G * VW:(qc + 1) * G * VW]
                dst = dst.rearrange("p (g c) -> p g c", c=VW)[:, :, 0:DF]
                src = v_ps[:].rearrange("p (g c) -> p g c", c=DF)
                # scatter on ACT, keeping DVE free so qg(0) runs early
                nc.scalar.activation(dst, src, ACT.Copy)

        # ---------- phase C: attention main loop ----------
        with ExitStack() as pcs:
            qg_pool = pcs.enter_context(tc.tile_pool(name="qg", bufs=2))
            bias_pool = pcs.enter_context(tc.tile_pool(name="bias", bufs=8))
            e_pool = pcs.enter_context(tc.tile_pool(name="epool", bufs=4))
            p8_pool = pcs.enter_context(tc.tile_pool(name="p8", bufs=5))
            nrm_pool = pcs.enter_context(tc.tile_pool(name="nrm", bufs=2))
            s_pool = pcs.enter_context(tc.tile_pool(name="sps", bufs=3, space="PSUM"))
            o_pool = pcs.enter_context(tc.tile_pool(name="ops", bufs=1, space="PSUM"))

            v_view = v_sb[:].rearrange("p (q g c) -> p q g c", q=QC, g=G)

            def make_qg(g):
                # Qg^T = Q^T * (Wtalk[g, h] / sqrt(DF)) -- folds the head mix
                qg_sb = qg_pool.tile([128, EC * PH], bf16, tag="qg")
                for ec in range(EC):
                    nc.vector.tensor_scalar_mul(
                        qg_sb[:, ec * PH:(ec + 1) * PH],
                        qt_sb[:, ec * PH:(ec + 1) * PH],
                        wt_sb[:, ec * G + g: ec * G + g + 1])
                return qg_sb

            qg_next = make_qg(0)
            for g in range(G):
                qg_sb = qg_next
                o_ps = o_pool.tile([VW, PH], f32, tag="ops")

                def av(p_tile, qc):
                    for pc in range(PH // 512):
                        nc.tensor.matmul(
                            o_ps[:, pc * 512:(pc + 1) * 512],
                            lhsT=v_view[:, qc, g, :],
                            rhs=p_tile[:, pc * 512:(pc + 1) * 512],
                            start=(qc == 0), stop=(qc == QC - 1))

                pend = []  # [(P_tile, qc)] awaiting their AV pass (skew 2)
                for qc in range(QC):
                    b_sb = bias_pool.tile([128, PH], bf16, tag="bias")
                    # last g's bias via scalar queue: sync drains its backlog
                    # before phase D's output DMAs need it
                    beng = nc.scalar if g == G - 1 else nc.sync
                    beng.dma_start(b_sb[:], biast[g, qc * 128:(qc + 1) * 128, :])
                    s_ps = s_pool.tile([128, PH], f32, tag="sps")
                    for ec in range(EC):
                        for pc in range(PH // 512):
                            nc.tensor.matmul(
                                s_ps[:, pc * 512:(pc + 1) * 512],
                                lhsT=kt_sb[:, ec * P + qc * 128: ec * P + (qc + 1) * 128],
                                rhs=qg_sb[:, ec * PH + pc * 512: ec * PH + (pc + 1) * 512],
                                start=(ec == 0), stop=(ec == EC - 1))
                    # P = exp(S + b) = exp(S - 2) * exp(b): ACT exp straight
                    # from PSUM, DVE all-SBUF multiply (2x mode eligible);
                    # the -2 shift cancels in normalization.
                    e_sb = e_pool.tile([128, PH], bf16, tag="exp")
                    nc.scalar.activation(e_sb[:], s_ps[:], ACT.Exp, bias=neg2_sb[:])
                    p_sb = p8_pool.tile([128, PH], bf16, tag="prob")
                    nc.vector.tensor_mul(p_sb[:], e_sb[:], b_sb[:])
                    if qc == 8 and g + 1 < G:
                        # mid-g so DVE finishes it well before S(g+1) needs it
                        qg_next = make_qg(g + 1)
                    pend.append((p_sb, qc))
                    if len(pend) > 2:
                        av(*pend.pop(0))
                for pt, pq in pend:
                    av(pt, pq)
                # evacuate o_ps to SBUF in one op (frees the PSUM slot fast),
                # then normalize off-PSUM: out^T[df, p] / sums[p]
                po = (g % 2) * DF
                if g < G - 1:
                    # evacuate o_ps to SBUF in one op (frees the PSUM slot
                    # fast), then normalize off-PSUM: out^T[df, p] / sums[p]
                    o_sb = nrm_pool.tile([VW, PH], f32, tag="osb")
                    nc.scalar.activation(o_sb[:], o_ps[:], ACT.Copy)
                    sum_sb = nrm_pool.tile([1, PH], f32, tag="sum")
                    nc.scalar.activation(sum_sb[:], o_ps[DF:DF + 1, :], ACT.Copy)
                    r_sb = nrm_pool.tile([1, PH], f32, tag="r")
                    nc.vector.reciprocal_approx_fast(r_sb[:], sum_sb[:])
                    rb_sb = nrm_pool.tile([DF, PH], f32, tag="rb")
                    nc.gpsimd.partition_broadcast(rb_sb[:], r_sb[:])
                    nc.vector.tensor_mul(
                        ocat_sbs[g // 2][po:po + DF, :], o_sb[0:DF, :], rb_sb[:])
                else:
                    # last g gates phase D: pipeline the normalize in two
                    # 512-col halves so out-proj's ec3 steps unblock earlier
                    o_sb = nrm_pool.tile([VW, PH], f32, tag="osb")
                    for ch in range(2):
                        sl = slice(ch * 512, (ch + 1) * 512)
                        sum_c = nrm_pool.tile([1, 512], f32, tag="sum")
                        nc.scalar.activation(sum_c[:], o_ps[DF:DF + 1, sl],
                                             ACT.Copy)
                        r_c = nrm_pool.tile([1, 512], f32, tag="r")
                        nc.vector.reciprocal_approx_fast(r_c[:], sum_c[:])
                        rb_c = nrm_pool.tile([DF, 512], f32, tag="rb")
                        nc.gpsimd.partition_broadcast(rb_c[:], r_c[:])
                        if ch == 0:
                            nc.scalar.activation(o_sb[:], o_ps[:], ACT.Copy)
                        nc.vector.tensor_mul(
                            ocat_sbs[g // 2][po:po + DF, sl],
                            o_sb[0:DF, sl], rb_c[:])

        # ---------- phase D: output projection ----------
        with ExitStack() as pd:
            y_pool = pd.enter_context(tc.tile_pool(name="yps", bufs=4, space="PSUM"))
            ysb_pool = pd.enter_context(tc.tile_pool(name="ysb", bufs=4))
            for pc in range(PH // 128):
                y_ps = y_pool.tile([128, D], f32, tag="yps")
                for ec in range(EC):
                    nc.tensor.matmul(
                        y_ps[:],
                        lhsT=ocat_sbs[ec][:, pc * 128:(pc + 1) * 128],
                        rhs=wo_sb[:, ec * D:(ec + 1) * D],
                        start=(ec == 0), stop=(ec == EC - 1))
                y_sb = ysb_pool.tile([128, D], bf16, tag="ysb")
                nc.scalar.activation(y_sb[:], y_ps[:], ACT.Copy)
                # two queues so the final output drain halves
                dma_eng = nc.sync if pc % 2 == 0 else nc.scalar
                dma_eng.dma_start(y[pc * 128:(pc + 1) * 128, :], y_sb[:])

    nc.compile()
    return nc


def kernel(x, attn_bias, Wq, Wk, Wv, Wtalk, Wo, **trace_kwargs):
    global LAST_RESULTS
    from concourse.bass_utils import run_bass_kernel_spmd

    x = np.asarray(x, dtype=np.float32)
    attn_bias = np.asarray(attn_bias, dtype=np.float32)
    Wq = np.asarray(Wq, dtype=np.float32)
    Wk = np.asarray(Wk, dtype=np.float32)
    Wv = np.asarray(Wv, dtype=np.float32)
    Wtalk = np.asarray(Wtalk, dtype=np.float32)
    Wo = np.asarray(Wo, dtype=np.float32)

    if "nc" not in _CACHE:
        _CACHE["nc"] = _build_program()
    nc = _CACHE["nc"]

    # host-side layout prep (cheap, reused across cores)
    import ml_dtypes
    bf = ml_dtypes.bfloat16
    xts = [np.ascontiguousarray(x[b].T).astype(bf) for b in range(B)]      # [D, P]
    xqts = [[np.ascontiguousarray(x[b, s * PH:(s + 1) * PH, :].T).astype(bf)
             for s in range(2)] for b in range(B)]                         # [D, PH]
    # expb = exp(bias^T): the kernel multiplies exp(S-2) by this (bf16)
    biasts = [np.ascontiguousarray(
        np.exp(attn_bias[0, :, s * PH:(s + 1) * PH, :].transpose(0, 2, 1)))
        .astype(bf) for s in range(2)]
    wt = np.ascontiguousarray((np.repeat(Wtalk, DF, axis=1) / np.sqrt(DF)).T
                              .astype(np.float32))                         # [512, 8]
    wq8, wk8, wv8, wo8 = (w.astype(bf) for w in (Wq, Wk, Wv, Wo))

    in_maps = []
    for c in range(N_CORES):
        b, s = c // 2, c % 2
        in_maps.append({
            "xt": xts[b], "xqt": xqts[b][s], "biast": biasts[s],
            "wq": wq8, "wk": wk8, "wv": wv8, "wo": wo8, "wt": wt,
        })

    res = run_bass_kernel_spmd(nc, in_maps, list(range(N_CORES)), **trace_kwargs)
    LAST_RESULTS = res

    out = np.empty((B, P, D), dtype=np.float32)
    for c in range(N_CORES):
        b, s = c // 2, c % 2
        out[b, s * PH:(s + 1) * PH, :] = np.asarray(
            res.results[c]["y"]).astype(np.float32)
    return out


# revision 80
# speedup vs baseline: 1.1925x; 1.1925x over previous
"""Trainium2 Bass kernel for a talking-heads MHSA block.

Reference computation (B=4, P=2048, D=512, H=8, DF=64, fp32):
    q = (x @ Wq) / sqrt(DF);  k = x @ Wk;  v = x @ Wv      (per-head reshape)
    attn[b,h]   = q_h k_h^T
    attn2[b,g]  = sum_h Wtalk[g,h] attn[b,h]               (talking heads)
    P           = softmax(attn2 + bias, axis=-1)
    out         = concat_g(P_g v_g) @ Wo

Sharding: 8 cores, data-parallel: core c -> batch b=c//2, query-half s=c%2
(1024 query rows, all heads, full 2048 keys). No collectives.

Per-core algorithm (all-bf16 matmuls, fp32 accumulation, no transposes):
  - host pre-casts inputs to bf16, pre-transposes x -> x^T, and precomputes
    expb = exp(bias^T) in bf16
  - talking-heads mix folded into Q: S_mixed[g] = (Wtalk[g,h]/8 * Q) . K
    over all 512 features -> dense 512-deep bf16 matmul. (fp8 DoubleRow was
    measured 2x faster on the PE but fails the 2e-2 gate: e4m3 operand
    rounding gives ~4% logit noise -> rel err 4.7e-2; fp8 probs/V in AV
    give 3.2e-2. The talking-heads 8x redundancy is PE-conserved: per-head
    QK + on-chip mix needs a partition interleave no engine does cheaply.)
  - softmax via exp(S+b) = exp(S-2) * exp(b): ACT computes E=exp(S-2)
    straight from PSUM, DVE multiplies by expb (all-SBUF op, 2x DVE mode);
    the -2 shift cancels in normalization
  - AV bf16 with a ones-column in V' producing denominators for free; AV
    emission skewed 2 key-chunks behind the S-matmuls so the PE never waits
    on exp; qg(g+1) computed mid-g on DVE
  - normalization after AV; out^T blocks (split per g-pair so phase D deps
    are per-block) feed the output projection as lhsT directly.
"""
import sys
from contextlib import ExitStack

import numpy as np

if "/opt/trn_rl_repo" not in sys.path:
    sys.path.insert(0, "/opt/trn_rl_repo")

B, P, D = 4, 2048, 512
H, DF = 8, 64
G = H                 # output head groups
PH = P // 2           # query rows per core
DC = D // 128         # 4 contraction chunks for d
EC = (H * DF) // 128  # 4 chunks for e = (h, df)
QC = P // 128         # 16 key chunks
VW = DF + 4           # V' width per group: 64 cols of V + ones column + pad
                      # (dual-fp8 LoadWeights needs 4-byte-aligned tiles)
N_CORES = 8

_CACHE = {}
LAST_RESULTS = None


def _build_program():
    import concourse.mybir as mybir
    import concourse.tile as tile
    from concourse import bacc

    f32 = mybir.dt.float32
    bf16 = mybir.dt.bfloat16
    fp8 = mybir.dt.float8e4
    ACT = mybir.ActivationFunctionType
    DR = mybir.MatmulPerfMode.DoubleRow

    nc = bacc.Bacc("TRN2", target_bir_lowering=False, debug=False)
    xt = nc.dram_tensor("xt", [D, P], bf16, kind="ExternalInput").ap()
    xqt = nc.dram_tensor("xqt", [D, PH], bf16, kind="ExternalInput").ap()
    biast = nc.dram_tensor("biast", [G, P, PH], bf16, kind="ExternalInput").ap()
    wq = nc.dram_tensor("wq", [D, H * DF], bf16, kind="ExternalInput").ap()
    wk = nc.dram_tensor("wk", [D, H * DF], bf16, kind="ExternalInput").ap()
    wv = nc.dram_tensor("wv", [D, H * DF], bf16, kind="ExternalInput").ap()
    wo = nc.dram_tensor("wo", [H * DF, D], bf16, kind="ExternalInput").ap()
    wt = nc.dram_tensor("wt", [H * DF, G], f32, kind="ExternalInput").ap()
    y = nc.dram_tensor("y", [PH, D], bf16, kind="ExternalOutput").ap()

    with tile.TileContext(nc) as tc, ExitStack() as ctx:
        persist = ctx.enter_context(tc.tile_pool(name="persist", bufs=1))
        qt_sb = persist.tile([128, EC * PH], bf16, tag="qt")      # Q^T [e, p]
        kt_sb = persist.tile([128, EC * P], bf16, tag="kt")       # K^T [e, q]
        v_sb = persist.tile([128, QC * G * VW], bf16, tag="v")    # V' [q, g*VW+c]
        wo_sb = persist.tile([128, EC * D], bf16, tag="wo")
        wt_sb = persist.tile([128, EC * G], f32, tag="wt")
        # out^T [e, p] split per g-pair so phase D deps are per-block
        ocat_sbs = [persist.tile([128, PH], bf16, name=f"ocat{ec}", tag=f"ocat{ec}")
                    for ec in range(EC)]
        neg2_sb = persist.tile([128, 1], f32, tag="neg2")
        nc.gpsimd.memset(neg2_sb[:], -2.0)

        def bf_load(dst_tile, dram_ap, n, m):
            # bf16 DRAM -> SBUF, one descriptor per 128-partition chunk
            nc.gpsimd.dma_start(
                dst_tile[:].rearrange("p (c m) -> p c m", c=n),
                dram_ap.rearrange("(c p) m -> p c m", p=128))

        # ---------- phase B: staging + QKV projections ----------
        with ExitStack() as pb:
            stage = pb.enter_context(tc.tile_pool(name="stage", bufs=1))
            xt_sb = stage.tile([128, DC * P], bf16, tag="xt")
            xqt_sb = stage.tile([128, DC * PH], bf16, tag="xqt")
            wq_sb = stage.tile([128, DC * D], bf16, tag="wq")
            wk_sb = stage.tile([128, DC * D], bf16, tag="wk")
            wv_sb = stage.tile([128, DC * D], bf16, tag="wv")
            # Q-proj inputs split across BOTH queues in parallel (wq on
            # gpsimd, xqt on sync) so the first matmul starts earliest
            for dc in range(DC):
                nc.gpsimd.dma_start(wq_sb[:, dc * D:(dc + 1) * D],
                                    wq[dc * 128:(dc + 1) * 128, :])
            for dc in range(DC):
                nc.sync.dma_start(xqt_sb[:, dc * PH:(dc + 1) * PH],
                                  xqt[dc * 128:(dc + 1) * 128, :])
            for dc in range(DC):
                nc.sync.dma_start(wk_sb[:, dc * D:(dc + 1) * D],
                                  wk[dc * 128:(dc + 1) * 128, :])
            for dc in range(DC):
                nc.gpsimd.dma_start(xt_sb[:, dc * P:(dc + 1) * P],
                                    xt[dc * 128:(dc + 1) * 128, :])
            bf_load(wv_sb, wv, DC, D)
            bf_load(wo_sb, wo, EC, D)
            nc.sync.dma_start(
                wt_sb[:].rearrange("p (c m) -> p c m", c=EC),
                wt.rearrange("(c p) m -> p c m", p=128))

            nc.gpsimd.memset(v_sb[:], 1.0)  # ones columns of V'

            psA = pb.enter_context(tc.tile_pool(name="psA", bufs=2, space="PSUM"))
            psB = pb.enter_context(tc.tile_pool(name="psB", bufs=4, space="PSUM"))

            # Q^T[e, p] = Wq^T x^T (query half only)
            for ec in range(EC):
                q_ps = psA.tile([128, PH], f32, tag="qps")
                for pc in range(PH // 512):
                    for dc in range(DC):
                        nc.tensor.matmul(
                            q_ps[:, pc * 512:(pc + 1) * 512],
                            lhsT=wq_sb[:, dc * D + ec * 128: dc * D + (ec + 1) * 128],
                            rhs=xqt_sb[:, dc * PH + pc * 512: dc * PH + (pc + 1) * 512],
                            start=(dc == 0), stop=(dc == DC - 1))
                nc.scalar.activation(qt_sb[:, ec * PH:(ec + 1) * PH], q_ps[:], ACT.Copy)
            # K^T[e, q] over all keys (qn outer: early key-chunks complete
            # first so phase C's S-matmuls can begin before V-proj drains)
            for qn in range(P // 512):
                for ec in range(EC):
                    k_ps = psB.tile([128, 512], f32, tag="kvps")
                    for dc in range(DC):
                        nc.tensor.matmul(
                            k_ps[:],
                            lhsT=wk_sb[:, dc * D + ec * 128: dc * D + (ec + 1) * 128],
                            rhs=xt_sb[:, dc * P + qn * 512: dc * P + (qn + 1) * 512],
                            start=(dc == 0), stop=(dc == DC - 1))
                    nc.scalar.activation(
                        kt_sb[:, ec * P + qn * 512: ec * P + (qn + 1) * 512],
                        k_ps[:], ACT.Copy)
            # V[q, e] natural layout, scattered into V' with ones columns
            for qc in range(QC):
                v_ps = psB.tile([128, 512], f32, tag="kvps")
                for dc in range(DC):
                    nc.tensor.matmul(
                        v_ps[:],
                        lhsT=xt_sb[:, dc * P + qc * 128: dc * P + (qc + 1) * 128],
                        rhs=wv_sb[:, dc * D:(dc + 1) * D],
                        start=(dc == 0), stop=(dc == DC - 1))
                dst = v_sb[:, qc * G * VW:(qc + 1) * G * VW]
                dst = dst.rearrange("p (g c) -> p g c", c=VW)[:, :, 0:DF]
                src = v_ps[:].rearrange("p (g c) -> p g c", c=DF)
                # scatter on ACT, keeping DVE free so qg(0) runs early
                nc.scalar.activation(dst, src, ACT.Copy)

        # ---------- phase C: attention main loop ----------
        with ExitStack() as pcs:
            qg_pool = pcs.enter_context(tc.tile_pool(name="qg", bufs=2))
            bias_pool = pcs.enter_context(tc.tile_pool(name="bias", bufs=8))
            e_pool = pcs.enter_context(tc.tile_pool(name="epool", bufs=4))
            p8_pool = pcs.enter_context(tc.tile_pool(name="p8", bufs=5))
            nrm_pool = pcs.enter_context(tc.tile_pool(name="nrm", bufs=2))
            s_pool = pcs.enter_context(tc.tile_pool(name="sps", bufs=3, space="PSUM"))
            o_pool = pcs.enter_context(tc.tile_pool(name="ops", bufs=1, space="PSUM"))

            v_view = v_sb[:].rearrange("p (q g c) -> p q g c", q=QC, g=G)

            def make_qg(g):
                # Qg^T = Q^T * (Wtalk[g, h] / sqrt(DF)) -- folds the head mix
                qg_sb = qg_pool.tile([128, EC * PH], bf16, tag="qg")
                for ec in range(EC):
                    nc.vector.tensor_scalar_mul(
                        qg_sb[:, ec * PH:(ec + 1) * PH],
                        qt_sb[:, ec * PH:(ec + 1) * PH],
                        wt_sb[:, ec * G + g: ec * G + g + 1])
                return qg_sb

            qg_next = make_qg(0)
            for g in range(G):
                qg_sb = qg_next
                o_ps = o_pool.tile([VW, PH], f32, tag="ops")

                def av(p_tile, qc):
                    for pc in range(PH // 512):
                        nc.tensor.matmul(
                            o_ps[:, pc * 512:(pc + 1) * 512],
                            lhsT=v_view[:, qc, g, :],
                            rhs=p_tile[:, pc * 512:(pc + 1) * 512],
                            start=(qc == 0), stop=(qc == QC - 1))

                pend = []  # [(P_tile, qc)] awaiting their AV pass (skew 2)
                for qc in range(QC):
                    b_sb = bias_pool.tile([128, PH], bf16, tag="bias")
                    nc.sync.dma_start(b_sb[:], biast[g, qc * 128:(qc + 1) * 128, :])
                    s_ps = s_pool.tile([128, PH], f32, tag="sps")
                    for ec in range(EC):
                        for pc in range(PH // 512):
                            nc.tensor.matmul(
                                s_ps[:, pc * 512:(pc + 1) * 512],
                                lhsT=kt_sb[:, ec * P + qc * 128: ec * P + (qc + 1) * 128],
                                rhs=qg_sb[:, ec * PH + pc * 512: ec * PH + (pc + 1) * 512],
                                start=(ec == 0), stop=(ec == EC - 1))
                    # P = exp(S + b) = exp(S - 2) * exp(b): ACT exp straight
                    # from PSUM, DVE all-SBUF multiply (2x mode eligible);
                    # the -2 shift cancels in normalization.
                    e_sb = e_pool.tile([128, PH], bf16, tag="exp")
                    nc.scalar.activation(e_sb[:], s_ps[:], ACT.Exp, bias=neg2_sb[:])
                    p_sb = p8_pool.tile([128, PH], bf16, tag="prob")
                    nc.vector.tensor_mul(p_sb[:], e_sb[:], b_sb[:])
                    if qc == 8 and g + 1 < G:
                        # mid-g so DVE finishes it well before S(g+1) needs it
                        qg_next = make_qg(g + 1)
                    pend.append((p_sb, qc))
                    if len(pend) > 2:
                        av(*pend.pop(0))
                for pt, pq in pend:
                    av(pt, pq)
                # evacuate o_ps to SBUF in one op (frees the PSUM slot fast),
                # then normalize off-PSUM: out^T[df, p] / sums[p]
                po = (g % 2) * DF
                if g < G - 1:
                    # evacuate o_ps to SBUF in one op (frees the PSUM slot
                    # fast), then normalize off-PSUM: out^T[df, p] / sums[p]
                    o_sb = nrm_pool.tile([VW, PH], f32, tag="osb")
                    nc.scalar.activation(o_sb[:], o_ps[:], ACT.Copy)
                    sum_sb = nrm_pool.tile([1, PH], f32, tag="sum")
                    nc.scalar.activation(sum_sb[:], o_ps[DF:DF + 1, :], ACT.Copy)
                    r_sb = nrm_pool.tile([1, PH], f32, tag="r")
                    nc.vector.reciprocal_approx_fast(r_sb[:], sum_sb[:])
                    rb_sb = nrm_pool.tile([DF, PH], f32, tag="rb")
                    nc.gpsimd.partition_broadcast(rb_sb[:], r_sb[:])
                    nc.vector.tensor_mul(
                        ocat_sbs[g // 2][po:po + DF, :], o_sb[0:DF, :], rb_sb[:])
                else:
                    # last g gates phase D: pipeline the normalize in two
                    # 512-col halves so out-proj's ec3 steps unblock earlier
                    o_sb = nrm_pool.tile([VW, PH], f32, tag="osb")
                    for ch in range(2):
                        sl = slice(ch * 512, (ch + 1) * 512)
                        sum_c = nrm_pool.tile([1, 512], f32, tag="sum")
                        nc.scalar.activation(sum_c[:], o_ps[DF:DF + 1, sl],
                                             ACT.Copy)
                        r_c = nrm_pool.tile([1, 512], f32, tag="r")
                        nc.vector.reciprocal_approx_fast(r_c[:], sum_c[:])
                        rb_c = nrm_pool.tile([DF, 512], f32, tag="rb")
                        nc.gpsimd.partition_broadcast(rb_c[:], r_c[:])
                        if ch == 0:
                            nc.scalar.activation(o_sb[:], o_ps[:], ACT.Copy)
                        nc.vector.tensor_mul(
                            ocat_sbs[g // 2][po:po + DF, sl],
                            o_sb[0:DF, sl], rb_c[:])

        # ---------- phase D: output projection ----------
        with ExitStack() as pd:
            y_pool = pd.enter_context(tc.tile_pool(name="yps", bufs=4, space="PSUM"))
            ysb_pool = pd.enter_context(tc.tile_pool(name="ysb", bufs=4))
            for pc in range(PH // 128):
                y_ps = y_pool.tile([128, D], f32, tag="yps")
                for ec in range(EC):
                    nc.tensor.matmul(
                        y_ps[:],
                        lhsT=ocat_sbs[ec][:, pc * 128:(pc + 1) * 128],
                        rhs=wo_sb[:, ec * D:(ec + 1) * D],
                        start=(ec == 0), stop=(ec == EC - 1))
                y_sb = ysb_pool.tile([128, D], bf16, tag="ysb")
                nc.scalar.activation(y_sb[:], y_ps[:], ACT.Copy)
                # two queues so the final output drain halves
                dma_eng = nc.sync if pc % 2 == 0 else nc.scalar
                dma_eng.dma_start(y[pc * 128:(pc + 1) * 128, :], y_sb[:])

    nc.compile()
    return nc


def kernel(x, attn_bias, Wq, Wk, Wv, Wtalk, Wo, **trace_kwargs):
    global LAST_RESULTS
    from concourse.bass_utils import run_bass_kernel_spmd

    x = np.asarray(x, dtype=np.float32)
    attn_bias = np.asarray(attn_bias, dtype=np.float32)
    Wq = np.asarray(Wq, dtype=np.float32)
    Wk = np.asarray(Wk, dtype=np.float32)
    Wv = np.asarray(Wv, dtype=np.float32)
    Wtalk = np.asarray(Wtalk, dtype=np.float32)
    Wo = np.asarray(Wo, dtype=np.float32)

    if "nc" not in _CACHE:
        _CACHE["nc"] = _build_program()
    nc = _CACHE["nc"]

    # host-side layout prep (cheap, reused across cores)
    import ml_dtypes
    bf = ml_dtypes.bfloat16
    xts = [np.ascontiguousarray(x[b].T).astype(bf) for b in range(B)]      # [D, P]
    xqts = [[np.ascontiguousarray(x[b, s * PH:(s + 1) * PH, :].T).astype(bf)
             for s in range(2)] for b in range(B)]                         # [D, PH]
    # expb = exp(bias^T): the kernel multiplies exp(S-2) by this (bf16)
    biasts = [np.ascontiguousarray(
        np.exp(attn_bias[0, :, s * PH:(s + 1) * PH, :].transpose(0, 2, 1)))
        .astype(bf) for s in range(2)]
    wt = np.ascontiguousarray((np.repeat(Wtalk, DF, axis=1) / np.sqrt(DF)).T
                              .astype(np.float32))                         # [512, 8]
    wq8, wk8, wv8, wo8 = (w.astype(bf) for w in (Wq, Wk, Wv, Wo))

    in_maps = []
    for c in range(N_CORES):
        b, s = c // 2, c % 2
        in_maps.append({
            "xt": xts[b], "xqt": xqts[b][s], "biast": biasts[s],
            "wq": wq8, "wk": wk8, "wv": wv8, "wo": wo8, "wt": wt,
        })

    res = run_bass_kernel_spmd(nc, in_maps, list(range(N_CORES)), **trace_kwargs)
    LAST_RESULTS = res

    out = np.empty((B, P, D), dtype=np.float32)
    for c in range(N_CORES):
        b, s = c // 2, c % 2
        out[b, s * PH:(s + 1) * PH, :] = np.asarray(
            res.results[c]["y"]).astype(np.float32)
    return out
